# revision 58
# baseline (speedup 1.0000x reference)
"""BiMapGeo forward on 8 NeuronCores (TRN2, Bass/Tile).

P[b,o] = sum_c W[o,c]^T @ x[b,c] @ W[o,c]
  x: (256, 8, 128, 128) fp32 (symmetric in last two dims)
  W: (16, 8, 128, 64) fp32
  P: (256, 16, 64, 64) fp32

Sharding: data-parallel over batch (32 per core), W replicated.

Per-core kernel (default mode "v6"), per group of BG=4 batches, per c:
  [mm1 b0,b1][mm2(c-1) o0..7][mm1 b2,b3][mm2(c-1) o8..15]
  - mm1: M1[b,c] = x[b,c] @ W[:,c] (bf16, stationary=x[b,c] via symmetry,
    moving=W as 2x512). Each 512-col half goes to its OWN 1-bank PSUM
    tile (tags m1h0/m1h1, 2 bufs each = 4 banks) and is evicted as a
    single [128,512] fp32->bf16 copy (h0 on DVE, h1 on Act) that starts
    at half-completion. This h-granular rotation keeps every
    buffer-reuse deadline (~1.24us uniform spacing) above the ~1us
    effective eviction latency (copy + semaphore round trips) -- the
    binding constraint of the whole kernel: with whole-tile [128,1024]
    evictions any mm2 speedup is cancelled by PSUM-WAR rotation stalls.
  - mm2: 8-MM half-bursts, explicit 64-col tiled ldweights
    (tile_position (0,0)/(0,64)) + non-self-loading matmuls pinned in
    order with nosync dependency chains. This hides the 16x53ns weight
    loads under the streams (~107ns/MM = serial-stream rate; measured:
    in-kernel bursts never reach the 2x col-pair concurrency that
    isolated 16-MM bursts show, and 8-MM chunks never concur at all).
  - P accumulates across the c loop in 4x 1-bank PSUM tiles (2 o's per
    tile via partition packing); staged to SBUF on DVE/Act at c=0/1 of
    the next group, then DMA'd out.
Measured: 235.6us steady-state vs 272.5us for the previous interleaved
serial schedule (both at rel err 3.3e-3); serial-stream floor is ~219us.
"""

import os
import numpy as np
from contextlib import ExitStack

import concourse.bacc as bacc
import concourse.tile as tile
from concourse import mybir
from concourse.instruction_name_ordered_set import InstructionNameOrderedSet

MM2_MODE = os.environ.get("MM2_MODE", "v6")
ABL = os.environ.get("ABL", "")  # timing-only ablations: noP / noX / noevict

B_TOT, HI, HO, NI, NO = 256, 8, 16, 128, 64
NCORES = 8
B = B_TOT // NCORES  # 32 batches per core
BG = 4               # batches per group
NG = B // BG         # 8 groups
OQ = HO * NO         # 1024
HIH = HI // 2        # c-half size for x staging

F32 = mybir.dt.float32
BF16 = mybir.dt.bfloat16

_NC_CACHE = {}


def build_nc(loop_iters: int = 1, bodies: int = 1):
    nc = bacc.Bacc("TRN2", target_bir_lowering=False, debug=False)

    x_in = nc.dram_tensor("x", [B, HI, NI, NI], F32, kind="ExternalInput")
    w_in = nc.dram_tensor("W", [HO, HI, NI, NO], F32, kind="ExternalInput")
    p_out = nc.dram_tensor("P", [B, HO, NO, NO], F32, kind="ExternalOutput")

    with tile.TileContext(nc) as tc, ExitStack() as ctx:
        const = ctx.enter_context(tc.tile_pool(name="const", bufs=1))
        wstage = ctx.enter_context(tc.tile_pool(name="wstage", bufs=8))
        xstage = ctx.enter_context(tc.tile_pool(name="xstage", bufs=8))
        xpool = ctx.enter_context(tc.tile_pool(name="xpool", bufs=2))
        m1pool = ctx.enter_context(tc.tile_pool(name="m1pool", bufs=3))
        ppool = ctx.enter_context(tc.tile_pool(name="ppool", bufs=2))
        m1ps_pool = ctx.enter_context(tc.tile_pool(name="m1ps", bufs=2, space="PSUM"))
        pps_pool = ctx.enter_context(tc.tile_pool(name="pps", bufs=4, space="PSUM"))

        # W resident in SBUF as [j(128), c, o, q] bf16: moving operand for
        # mm1 (slices [j, 512]), stationary for mm2 (slices [i, 64]).
        w_bf = const.tile([NI, HI, HO, NO], BF16, tag="w_bf")

        def w_load(c):
            # Per (c, o-half) so mm1(c=0) only waits on a 256KB transfer.
            # Late c's ride the Pool SWDGE queue: descriptor generation on
            # the two HWDGE queues is the startup bottleneck (~1.6us per
            # half), and Pool is idle once the first x rounds are done.
            for h in range(2):
                w_st = wstage.tile([NI, HO // 2, NO], F32, tag="wst",
                                   name=f"wst{c}h{h}")
                if c >= 4:
                    q = nc.gpsimd
                elif c == 0:
                    q = nc.sync if h == 0 else nc.scalar
                else:
                    q = nc.sync if c % 2 == 0 else nc.scalar
                q.dma_start(
                    out=w_st[:],
                    in_=w_in[h * 8:(h + 1) * 8, c, :, :].transpose([1, 0, 2]),
                )
                eng = ("v", "a", "p", "v", "a", "p", "v", "a")[c]
                dst = w_bf[:, c, h * 8:(h + 1) * 8]
                if eng == "a":
                    nc.scalar.copy(dst, w_st[:])
                elif eng == "v":
                    nc.vector.tensor_copy(dst, w_st[:])
                else:
                    nc.gpsimd.tensor_copy(dst, w_st[:])

        w_load(0)

        def emit_body():
            emit_groups(nc, tc, x_in, p_out, w_bf, w_load,
                        xstage, xpool, m1pool, ppool, m1ps_pool, pps_pool)

        if loop_iters > 1:
            for c in range(1, HI):
                w_load(c)
            ET = mybir.EngineType
            with tc.For_i(0, loop_iters, 1, hint_engines=(ET.PE, ET.DVE, ET.Activation, ET.SP)):
                for _ in range(bodies):
                    emit_groups(nc, tc, x_in, p_out, w_bf, None,
                                xstage, xpool, m1pool, ppool, m1ps_pool, pps_pool)
        else:
            emit_body()
    nc.finalize()
    return nc


def emit_groups(nc, tc, x_in, p_out, w_bf, w_load_rest, xstage, xpool, m1pool, ppool, m1ps_pool, pps_pool):
    # x tile per group: [j(128), b, c, i] bf16; by symmetry usable as
    # [i, b, c, j]. DMA per (b, c-half) on SP, round to bf16 on Pool.
    # h-major order so the first c-half of every batch arrives first.
    def x_load(g, fine_b0=False):
        x_t = xpool.tile([NI, BG, HI, NI], BF16, tag="xt", name=f"xt{g}")
        if ABL == "noX" and g > 0:
            nc.gpsimd.memset(x_t[:, 0, 0, 0:2], 0.0)
            return x_t
        for h in range(2):
            for b in range(BG):
                if fine_b0 and b == 0:
                    # 2-c granularity for the first batch of group 0 so
                    # mm1(c=0,b=0) starts ~2.5us in instead of ~6.5us.
                    for s in range(2):
                        c0 = h * HIH + s * 2
                        x_sb = xstage.tile([NI, 2, NI], F32, tag="xstf",
                                           name=f"xstf{h}s{s}")
                        nc.sync.dma_start(
                            out=x_sb[:],
                            in_=x_in[g * BG, c0:c0 + 2].transpose([1, 0, 2]),
                        )
                        nc.gpsimd.tensor_copy(x_t[:, 0, c0:c0 + 2], x_sb[:])
                    continue
                x_sb = xstage.tile([NI, HIH, NI], F32, tag="xst",
                                   name=f"xst{g}b{b}h{h}")
                nc.sync.dma_start(
                    out=x_sb[:],
                    in_=x_in[g * BG + b, h * HIH:(h + 1) * HIH].transpose([1, 0, 2]),
                )
                nc.gpsimd.tensor_copy(x_t[:, b, h * HIH:(h + 1) * HIH], x_sb[:])
        return x_t

    mm2_chain = {"prev": None}

    def _chain(inst):
        # Pin PE-queue order: tile's scheduler ignores program order, so
        # explicit ldweights must be ordered w.r.t. the matmuls that use
        # (and the ones that previously used) the same array tile.
        if mm2_chain["prev"] is not None:
            s = InstructionNameOrderedSet()
            s.add(mm2_chain["prev"])
            inst.ins.add_nosync_dependencies_from(s)
        mm2_chain["prev"] = inst.ins.name
        return inst

    def mm2_part(pps_tiles, c, m1_c, half):
        # P[o] accumulation for one c: 16 matmuls of N=256, two o's per
        # PSUM tile column-block via tile_position row packing; emitted in
        # a 9-MM and a 7-MM batch so they pad the two m1_ps WAR windows
        # (DVE evictions need ~1292ns, Act ~1138ns).
        # Two o-pairs share each 2KB PSUM bank. start=True marks the whole
        # bank (per touched partition range) pending-zero, so only the
        # first column block (t%2==0) starts; block 1's first write then
        # lands on pending-zero bytes and overwrites correctly.
        # 12+4 split: the 12-MM first chunk keeps all eviction-rotation
        # deadlines (b2's WAR gains 4x107ns of slack; the boundary deps
        # retain ~120ns) and measured slightly faster than 8+8.
        S = int(os.environ.get("V6SPLIT", "12"))
        rng = list(range(0, 16) if half is None
                   else range(0, S) if half == 0 else range(S, 16))
        if MM2_MODE in ("ldw", "v6", "v7", "v8"):
            # Explicit tiled weight loads + non-self-loading matmuls:
            # LDW(T0), LDW(T1), MM(T0), MM(T1) per o-pair so the two MMs
            # (disjoint col-groups) can stream concurrently and the next
            # pair's LDWs overlap them.
            alt = os.environ.get("V6ALT") == "1"
            for j in range(0, len(rng), 2):
                ks = (rng[j], rng[j + 1])
                if not alt:
                    for k in ks:
                        t, ph = k // 2, k % 2
                        o = 2 * t + ph
                        _chain(nc.tensor.ldweights(
                            w_bf[:, c, o, :], tile_position=(0, ph * 64)))
                for k in ks:
                    t, ph = k // 2, k % 2
                    pt = pps_tiles[t // 2]
                    o = 2 * t + ph
                    if alt:
                        _chain(nc.tensor.ldweights(
                            w_bf[:, c, o, :], tile_position=(0, ph * 64)))
                    mm = nc.tensor.matmul(
                        pt[ph * 64:(ph + 1) * 64, t % 2, :],
                        w_bf[:, c, o, :],
                        m1_c[:, :, o * NO:(o + 1) * NO],
                        start=(c == 0 and t % 2 == 0),
                        stop=(c == HI - 1),
                        tile_position=(0, ph * 64),
                        skip_group_check=True,
                    )
                    mm.ins.ldweights = False
                    _chain(mm)
            return
        for k in rng:
            t, ph = k // 2, k % 2
            pt = pps_tiles[t // 2]
            o = 2 * t + ph
            nc.tensor.matmul(
                pt[ph * 64:(ph + 1) * 64, t % 2, :],
                w_bf[:, c, o, :],
                m1_c[:, :, o * NO:(o + 1) * NO],
                start=(c == 0 and t % 2 == 0),
                stop=(c == HI - 1),
                tile_position=(0, ph * 64),
                skip_group_check=True,
            )

    def mm1(x_t, c, b, m1_c):
        if MM2_MODE in ("v6", "v7", "v8"):
            # h-granular PSUM rotation: each 512-col half gets its own
            # 1-bank tile (tags m1h0/m1h1, 2 bufs each). Evictions start
            # at half-completion and are single [128,512] copies (DVE for
            # h0, Act for h1), so the reuse deadline is met even when the
            # mm2 halves shrink (concurrent ldw-pattern bursts).
            lhsT = x_t[:, b, c, :]
            psA = m1ps_pool.tile([NI, 512], F32, tag="m1h0", name="m1h0")  # noqa
            nc.tensor.matmul(psA[:, :], lhsT, w_bf[:, c, 0:8, :],
                             start=True, stop=True)
            psB = m1ps_pool.tile([NI, 512], F32, tag="m1h1", name="m1h1")
            mmB = nc.tensor.matmul(psB[:, :], lhsT, w_bf[:, c, 8:16, :],
                                   start=True, stop=True)
            # anchor mm2's explicit LDWs behind the mm1 stream (a floating
            # LDW would be clobbered by mm1's full-array stationary load)
            mm2_chain["prev"] = mmB.ins.name
            if ABL == "noevict":
                if b == 0:
                    nc.vector.tensor_copy(m1_c[:, 0, 0:2], psA[:, 0:2])
            elif MM2_MODE == "v7" and b >= 2:
                # quarter-split across both engines: ~halves the eviction
                # latency for the batches whose buffers are reused right
                # after the c boundary.
                nc.vector.tensor_copy(m1_c[:, b, 0:256], psA[:, 0:256])
                nc.scalar.copy(m1_c[:, b, 256:512], psA[:, 256:512])
                nc.vector.tensor_copy(m1_c[:, b, 512:768], psB[:, 0:256])
                nc.scalar.copy(m1_c[:, b, 768:OQ], psB[:, 256:512])
            elif b % 2 == 1:
                # alternate engine assignment per batch parity so DVE
                # (0.96GHz) and Act (1.2GHz) carry equal eviction loads
                nc.scalar.copy(m1_c[:, b, 0:512], psA[:, :])
                nc.vector.tensor_copy(m1_c[:, b, 512:OQ], psB[:, :])
            else:
                nc.vector.tensor_copy(m1_c[:, b, 0:512], psA[:, :])
                nc.scalar.copy(m1_c[:, b, 512:OQ], psB[:, :])
            return
        m1_ps = m1ps_pool.tile([NI, OQ], F32, tag="m1ps")
        lhsT = x_t[:, b, c, :]
        for h in range(2):
            nc.tensor.matmul(
                m1_ps[:, h * 512:(h + 1) * 512],
                lhsT,
                w_bf[:, c, h * 8:(h + 1) * 8, :],
                start=True,
                stop=True,
            )
        # One eviction copy per batch, alternating DVE/Act. Any finer split
        # (even h-aligned staggered halves) adds ~400ns effective per-copy
        # overhead and measured worse in every arrangement tried.
        if b % 2 == 0:
            nc.vector.tensor_copy(m1_c[:, b, :], m1_ps[:, :])
        else:
            nc.scalar.copy(m1_c[:, b, :], m1_ps[:, :])

    def evict_p_pair(p_sb, pps_tiles, b0, i, eng, dma_qs=None):
        if ABL == "noP":
            return
        # Copy accumulator pair-tile i (o-pairs 2i, 2i+1) to SBUF, then
        # one DMA per o-pair. GPSIMD can't read PSUM on HW, so these ride
        # DVE/Act; callers stage pairs across c=0/c=1 of the next group to
        # stay inside the per-c engine budgets. For the last group (p_sb
        # None) no mm2 follows, so DMA straight from PSUM and skip the
        # staging copy.
        if p_sb is not None:
            if eng == "v":
                nc.vector.tensor_copy(p_sb[:, 2 * i:2 * i + 2, :], pps_tiles[i][:])
            else:
                nc.scalar.copy(p_sb[:, 2 * i:2 * i + 2, :], pps_tiles[i][:])
        for k in range(2):
            t = 2 * i + k
            q = dma_qs[k] if dma_qs else nc.sync
            src = p_sb[:, t, :] if p_sb is not None else pps_tiles[i][:, k, :]
            q.dma_start(
                out=p_out[b0:b0 + BG, 2 * t:2 * t + 2].rearrange(
                    "b o p q -> (o p) b q"
                ),
                in_=src,
            )

    def mm2_all(pps_tiles, c, m1_c, start_c, stop_c):
        # Full mm2 for one c, contiguous (no mm1 interleave): explicit
        # 64-col tiled weight loads + non-self-loading matmuls in
        # LDW,LDW,MM,MM pairs. The two MMs of a pair stream concurrently
        # on col-groups 0-1/2-3 (~2x), and the next pair's LDWs overlap
        # them; measured 988ns for the 16 MMs vs 1707ns serial.
        use_ldw = MM2_MODE != "v2noldw"
        for j in range(0, 16, 2):
            if use_ldw:
                for k in (j, j + 1):
                    t, ph = k // 2, k % 2
                    o = 2 * t + ph
                    _chain(nc.tensor.ldweights(
                        w_bf[:, c, o, :], tile_position=(0, ph * 64)))
            for k in (j, j + 1):
                t, ph = k // 2, k % 2
                pt = pps_tiles[t // 2]
                o = 2 * t + ph
                mm = nc.tensor.matmul(
                    pt[ph * 64:(ph + 1) * 64, t % 2, :],
                    w_bf[:, c, o, :],
                    m1_c[:, :, o * NO:(o + 1) * NO],
                    start=(start_c and t % 2 == 0),
                    stop=stop_c,
                    tile_position=(0, ph * 64),
                    skip_group_check=True,
                )
                if use_ldw:
                    mm.ins.ldweights = False
                    _chain(mm)

    def mm1_v2(x_t, c, b, m1_c):
        # mm1 for one batch; evict the two 512-col halves concurrently on
        # DVE and Act so the m1_ps buffer frees ~450ns after the stop,
        # keeping the 2-buffer WAR rotation stall-free in the contiguous
        # mm1 phase.
        m1_ps = m1ps_pool.tile([NI, OQ], F32, tag="m1ps")
        lhsT = x_t[:, b, c, :]
        last = None
        for h in range(2):
            last = nc.tensor.matmul(
                m1_ps[:, h * 512:(h + 1) * 512],
                lhsT,
                w_bf[:, c, h * 8:(h + 1) * 8, :],
                start=True,
                stop=True,
            )
        if ABL == "noevict":
            if b == 0:
                nc.vector.tensor_copy(m1_c[:, 0, 0:2], m1_ps[:, 0:2])
        elif MM2_MODE == "v2e":
            if b % 2 == 0:
                nc.vector.tensor_copy(m1_c[:, b, :], m1_ps[:, :])
            else:
                nc.scalar.copy(m1_c[:, b, :], m1_ps[:, :])
        else:
            nc.vector.tensor_copy(m1_c[:, b, 0:512], m1_ps[:, 0:512])
            nc.scalar.copy(m1_c[:, b, 512:OQ], m1_ps[:, 512:OQ])
        # Anchor the next mm2 burst's LDWs after this phase's matmuls: an
        # unanchored (dep-free) LDW scheduled before/inside the mm1 phase
        # would be clobbered by mm1's full-array stationary load.
        mm2_chain["prev"] = last.ins.name

    if w_load_rest is not None:
        # PE warm-up: the clock gate (HAM) runs the array at half speed
        # until ~3.4us of sustained activity. Burn dummy matmuls on scratch
        # data during the initial DMA wait so the first real mm1s run warm.
        # Single-shot only -- inside the timing loop the PE never cools.
        warm_src = ppool.tile([NI, 512], BF16, tag="warmsrc")
        nc.gpsimd.memset(warm_src[:], 0.25)
        warm_ps = m1ps_pool.tile(
            [NI, 512], F32,
            tag=("m1h0" if MM2_MODE in ("v6", "v7", "v8") else "m1ps"))
        for i in range(6):
            nc.tensor.matmul(
                warm_ps[0:64, :],
                warm_src[:, 0:64],
                warm_src[:, :],
                start=True,
                stop=True,
            )
    x_tiles = {0: x_load(0, fine_b0=(w_load_rest is not None))}
    if w_load_rest is not None:
        for c in range(1, HI):
            w_load_rest(c)
    prev = None  # [pps_tiles, m1_tile_of_c7, b0, g, p_sb] pending mm2(c=7) + evict

    if MM2_MODE == "v8":
        # c-pair superperiods: 8 contiguous mm1 MMs (two c's interleaved at
        # batch-pair granularity -> uniform 4-bank psum-ring spacing), then
        # one 32-MM contiguous ldw-pattern mm2 burst for both c's (long
        # enough for pair concurrency to engage; boundary slack is the
        # whole burst). Half the mode switches of v6.
        for g in range(NG):
            b0 = g * BG
            if g + 1 < NG:
                x_tiles[g + 1] = x_load(g + 1)
            x_t = x_tiles.pop(g)

            pps_tiles = [
                pps_pool.tile([NI, 2, BG * NO], F32, tag="pps", name=f"pps{g}_{i}")
                for i in range(4)
            ]

            for ce in range(0, HI, 2):
                co = ce + 1
                m1_e = m1pool.tile([NI, BG, OQ], BF16, tag="m1", name="m1e")
                m1_o = m1pool.tile([NI, BG, OQ], BF16, tag="m1", name="m1o")
                mm1(x_t, ce, 0, m1_e)
                mm1(x_t, ce, 1, m1_e)
                mm1(x_t, co, 0, m1_o)
                mm1(x_t, co, 1, m1_o)
                if ce == 0 and prev is not None:
                    ppp, m1p, b0p, gp, p_sb = prev
                    evict_p_pair(p_sb, ppp, b0p, 2, "v")
                    evict_p_pair(p_sb, ppp, b0p, 3, "a")
                    prev = None
                mm1(x_t, ce, 2, m1_e)
                mm1(x_t, ce, 3, m1_e)
                mm1(x_t, co, 2, m1_o)
                mm1(x_t, co, 3, m1_o)
                mm2_part(pps_tiles, ce, m1_e, None)
                mm2_part(pps_tiles, co, m1_o, None)

            # group done: stop was on c=HI-1; stage pairs 0,1 now, pairs
            # 2,3 at the top of the next group (before its first burst).
            p_sb = ppool.tile([NI, HO // 2, BG * NO], F32, tag="psb",
                              name=f"psb{g}")
            if g + 1 < NG:
                evict_p_pair(p_sb, pps_tiles, b0, 0, "v")
                evict_p_pair(p_sb, pps_tiles, b0, 1, "a")
                prev = [pps_tiles, None, b0, g, p_sb]
            else:
                for i in range(4):
                    evict_p_pair(p_sb, pps_tiles, b0, i,
                                 "v" if i % 2 == 0 else "a",
                                 dma_qs=(nc.sync, nc.scalar))
        return

    if MM2_MODE == "v5":
        # Software-pipelined: mm2 runs at depth 2 (burst for c-2 between
        # mm1 b1 and b2 of c), so m1 evictions get a full extra c-window
        # of slack; single whole-tile evictions (cheapest) alternate
        # DVE/Act; P staging for group G happens between G's final burst
        # and G+1's first burst (pairs 0,1 after the final burst, pairs
        # 2,3 at the top of the next iteration).
        burst_q = []   # (pps_tiles, c, m1_c, start_c, stop_c, b0, g)
        pend = None    # {"pps","b0","g","p_sb","stage"}
        for g in range(NG):
            b0 = g * BG
            if g + 1 < NG:
                x_tiles[g + 1] = x_load(g + 1)
            x_t = x_tiles.pop(g)

            pps_tiles = [
                pps_pool.tile([NI, 2, BG * NO], F32, tag="pps", name=f"pps{g}_{i}")
                for i in range(4)
            ]

            for c in range(HI):
                if pend is not None and pend["stage"] == 1:
                    evict_p_pair(pend["p_sb"], pend["pps"], pend["b0"], 2, "v")
                    evict_p_pair(pend["p_sb"], pend["pps"], pend["b0"], 3, "a")
                    pend = None
                m1_c = m1pool.tile([NI, BG, OQ], BF16, tag="m1")
                mm1_v2(x_t, c, 0, m1_c)
                mm1_v2(x_t, c, 1, m1_c)
                if len(burst_q) == 2:
                    bd = burst_q.pop(0)
                    mm2_all(bd[0], bd[1], bd[2], bd[3], bd[4])
                    if bd[4]:  # final burst of its group: stage pairs 0,1
                        p_sb = ppool.tile([NI, HO // 2, BG * NO], F32,
                                          tag="psb", name=f"psb{bd[6]}")
                        evict_p_pair(p_sb, bd[0], bd[5], 0, "v")
                        evict_p_pair(p_sb, bd[0], bd[5], 1, "a")
                        pend = {"pps": bd[0], "b0": bd[5], "g": bd[6],
                                "p_sb": p_sb, "stage": 1}
                mm1_v2(x_t, c, 2, m1_c)
                mm1_v2(x_t, c, 3, m1_c)
                burst_q.append(
                    (pps_tiles, c, m1_c, c == 0, c == HI - 1, b0, g))

        # tail: flush the last two bursts and evict group NG-1 direct.
        for bd in burst_q:
            mm2_all(bd[0], bd[1], bd[2], bd[3], bd[4])
        ppp, b0p, gp = pps_tiles, (NG - 1) * BG, NG - 1
        p_sb = ppool.tile([NI, HO // 2, BG * NO], F32, tag="psb", name=f"psb{gp}")
        for i in range(4):
            evict_p_pair(p_sb, ppp, b0p, i, "v" if i % 2 == 0 else "a",
                         dma_qs=(nc.sync, nc.scalar))
        return

    if MM2_MODE in ("v2", "v2e", "v2noldw", "v4"):
        for g in range(NG):
            b0 = g * BG
            if g + 1 < NG:
                x_tiles[g + 1] = x_load(g + 1)
            x_t = x_tiles.pop(g)

            pps_tiles = [
                pps_pool.tile([NI, 2, BG * NO], F32, tag="pps", name=f"pps{g}_{i}")
                for i in range(4)
            ]

            m1_prev = None
            for c in range(HI):
                m1_c = m1pool.tile([NI, BG, OQ], BF16, tag="m1")
                nb1 = 2 if MM2_MODE == "v4" else BG
                for b in range(nb1):
                    mm1_v2(x_t, c, b, m1_c)
                if c > 0:
                    mm2_all(pps_tiles, c - 1, m1_prev,
                            start_c=(c == 1), stop_c=(c - 1 == HI - 1))
                elif prev is not None:
                    mm2_all(prev[0], HI - 1, prev[1],
                            start_c=False, stop_c=True)
                for b in range(nb1, BG):
                    mm1_v2(x_t, c, b, m1_c)
                if c == 0 and prev is not None:
                    ppp, m1p, b0p, gp, _ = prev
                    p_sb = ppool.tile([NI, HO // 2, BG * NO], F32, tag="psb",
                                      name=f"psb{gp}")
                    prev[4] = p_sb
                    evict_p_pair(p_sb, ppp, b0p, 0, "v")
                    evict_p_pair(p_sb, ppp, b0p, 1, "a")
                elif c == 1 and prev is not None:
                    ppp, m1p, b0p, gp, p_sb = prev
                    evict_p_pair(p_sb, ppp, b0p, 2, "v")
                    evict_p_pair(p_sb, ppp, b0p, 3, "a")
                    prev = None
                m1_prev = m1_c

            prev = [pps_tiles, m1_prev, b0, g, None]

        ppp, m1p, b0p, gp, _ = prev
        mm2_all(ppp, HI - 1, m1p, start_c=False, stop_c=True)
        p_sb = ppool.tile([NI, HO // 2, BG * NO], F32, tag="psb", name=f"psb{gp}")
        for i in range(4):
            evict_p_pair(p_sb, ppp, b0p, i, "v" if i % 2 == 0 else "a",
                         dma_qs=(nc.sync, nc.scalar))
        return

    for g in range(NG):
        b0 = g * BG
        if g + 1 < NG:
            x_tiles[g + 1] = x_load(g + 1)
        x_t = x_tiles.pop(g)

        pps_tiles = [
            pps_pool.tile([NI, 2, BG * NO], F32, tag="pps", name=f"pps{g}_{i}")
            for i in range(4)
        ]

        m1_prev = None
        for c in range(HI):
            m1_c = m1pool.tile([NI, BG, OQ], BF16, tag="m1")
            mm1(x_t, c, 0, m1_c)
            mm1(x_t, c, 1, m1_c)
            h0 = None if MM2_MODE == "v7" else 0
            if MM2_MODE == "v7" and c == 1 and prev is not None:
                # stage pairs 2,3 BEFORE the full burst overwrites them
                ppp, m1p, b0p, gp, p_sb = prev
                evict_p_pair(p_sb, ppp, b0p, 2, "v")
                evict_p_pair(p_sb, ppp, b0p, 3, "a")
            if c > 0:
                mm2_part(pps_tiles, c - 1, m1_prev, h0)
            elif prev is not None:
                mm2_part(prev[0], HI - 1, prev[1], h0)
            if c == 0 and prev is not None:
                # P eviction of the previous group, pair tiles 0,1 (o 0..3):
                # emitted here (after their stops in mm2(c7) h0) so they sit
                # between m1-evict halves in the Act FIFO without starving
                # the WAR chains.
                ppp, m1p, b0p, gp, _ = prev
                p_sb = ppool.tile([NI, HO // 2, BG * NO], F32, tag="psb",
                                  name=f"psb{gp}")
                prev[4] = p_sb
                # P-output DMAs stay on the sync queue: routing them to the
                # scalar HWDGE measured +75us (strided descriptors stall the
                # Act-issued queue).
                evict_p_pair(p_sb, ppp, b0p, 0, "v")
                evict_p_pair(p_sb, ppp, b0p, 1, "a")
            elif c == 1 and prev is not None and MM2_MODE != "v7":
                ppp, m1p, b0p, gp, p_sb = prev
                evict_p_pair(p_sb, ppp, b0p, 2, "v")
                evict_p_pair(p_sb, ppp, b0p, 3, "a")
            mm1(x_t, c, 2, m1_c)
            mm1(x_t, c, 3, m1_c)
            if MM2_MODE != "v7":
                if c > 0:
                    mm2_part(pps_tiles, c - 1, m1_prev, 1)
                elif prev is not None:
                    mm2_part(prev[0], HI - 1, prev[1], 1)
            if c == 1 and prev is not None:
                prev = None
            m1_prev = m1_c

        prev = [pps_tiles, m1_prev, b0, g, None]

    ppp, m1p, b0p, gp, _ = prev
    if MM2_MODE == "v7":
        mm2_part(ppp, HI - 1, m1p, None)
    else:
        mm2_part(ppp, HI - 1, m1p, 0)
        mm2_part(ppp, HI - 1, m1p, 1)
    # Stage the last group through SBUF too (not direct-from-PSUM DMA): in
    # the steady loop the next body's first burst reuses these accumulator
    # banks, and a ~128KB PSUM-source DMA chain would gate it for ~8us.
    p_sb = ppool.tile([NI, HO // 2, BG * NO], F32, tag="psb", name=f"psb{gp}")
    for i in range(4):
        evict_p_pair(p_sb, ppp, b0p, i, "v" if i % 2 == 0 else "a",
                     dma_qs=(nc.sync, nc.scalar))


def kernel(x: np.ndarray, W: np.ndarray) -> np.ndarray:
    from concourse.bass_utils import run_bass_kernel_spmd

    x = np.ascontiguousarray(x, dtype=np.float32)
    W = np.ascontiguousarray(W, dtype=np.float32)

    if "nc" not in _NC_CACHE:
        _NC_CACHE["nc"] = build_nc()
    nc = _NC_CACHE["nc"]

    in_maps = [
        {"x": x[i * B : (i + 1) * B], "W": W} for i in range(NCORES)
    ]
    res = run_bass_kernel_spmd(nc, in_maps, list(range(NCORES)))
    out = np.concatenate([res.results[i]["P"] for i in range(NCORES)], axis=0)
    return out

